# revision 1
# baseline (speedup 1.0000x reference)
"""Multi-head self-attention (RoPE + causal) Trainium2 Bass kernel, 8-core SPMD.

Problem: B=2, L=2048, D=1024, H=16 heads of Dh=64.
Sharding: each core owns 2 heads x both batches (32 (b,h) pairs / 8 cores = 4).
  - qkv projection: column-parallel (each core computes q/k/v only for its heads)
  - attention: fully local per (b, head)
  - o_proj: row-parallel (each core contracts its 128 ctx dims); host sums the
    8 partial outputs (the unshard step for row-parallel sharding).

Layouts (T = transposed, feature-on-partition):
  xt   (B, 8, 128, L)   x^T chunks: partition=d-model chunk, free=L
  q/k  computed as qT (128=2 heads x 64, L); RoPE applied in this layout
  v    transposed on PE to (L-tile, dh) layout with an appended ones column,
       so the PV matmul also produces softmax row sums (flash-style)
  scores S^T (k-tile 128, q-block 512) per head; exp on ACT; causal masking via
       gpsimd affine_select on the post-exp probabilities (fill 0.0)
  ctx  (65, 512) PSUM accumulator per head (row 64 = softmax sums)
  out  (B, 8, 128, L) partial o_proj output (d-model on partition)

Matmul dtype: float32r (TF32-like, 1 cycle/row at N>=256; measured frob rel
err ~1.5e-4 on K=1024 GEMM vs 2.3e-3 for bf16).
"""
import os
import sys

import numpy as np

sys.path.insert(0, "/opt/trn_rl_repo")

import concourse.bass as bass  # noqa: E402
import concourse.mybir as mybir  # noqa: E402
import concourse.tile as tile  # noqa: E402
from concourse import bacc  # noqa: E402
from concourse.bass_utils import run_bass_kernel_spmd  # noqa: E402

B, L, D, H, DH = 2, 2048, 1024, 16, 64
NCORES = 8
KC = D // 128          # 8 contraction chunks for the projections
LT = L // 512          # 4 l-blocks of 512
NQB = L // 512         # 4 q-blocks of 512
NKT = L // 128         # 16 k-tiles of 128
F32 = mybir.dt.float32
F32R = mybir.dt.float32r

_BUILT = None


def build():
    nc = bacc.Bacc("TRN2", target_bir_lowering=False, debug=False,
                   num_devices=NCORES)

    xt_d = nc.dram_tensor("xt", [B, KC, 128, L], F32R, kind="ExternalInput")
    wq_d = nc.dram_tensor("wq", [KC, 128, 128], F32R, kind="ExternalInput")
    wk_d = nc.dram_tensor("wk", [KC, 128, 128], F32R, kind="ExternalInput")
    wv_d = nc.dram_tensor("wv", [KC, 128, 128], F32R, kind="ExternalInput")
    wo_d = nc.dram_tensor("wo", [128, KC, 128], F32R, kind="ExternalInput")
    swp_d = nc.dram_tensor("swp", [128, 128], F32R, kind="ExternalInput")
    cos_d = nc.dram_tensor("cosT", [128, L], F32, kind="ExternalInput")
    sin_d = nc.dram_tensor("sinT", [128, L], F32, kind="ExternalInput")
    ident_d = nc.dram_tensor("ident", [128, 128], F32, kind="ExternalInput")
    bqkv_d = nc.dram_tensor("bqkv", [128, 3], F32, kind="ExternalInput")
    bo_d = nc.dram_tensor("bo", [128, KC], F32, kind="ExternalInput")
    out_d = nc.dram_tensor("out", [B, KC, 128, L], F32, kind="ExternalOutput")

    with tile.TileContext(nc) as tc:
        with (
            tc.tile_pool(name="const", bufs=1) as constp,
            tc.tile_pool(name="x", bufs=KC) as xp,
            tc.tile_pool(name="qkv", bufs=1) as qkvp,
            tc.tile_pool(name="rope", bufs=1) as ropep,
            tc.tile_pool(name="vsb", bufs=1) as vsbp,
            tc.tile_pool(name="p", bufs=4) as pp,
            tc.tile_pool(name="work", bufs=2) as workp,
            tc.tile_pool(name="c2", bufs=5) as c2p,
            tc.tile_pool(name="outp", bufs=4) as outp,
            tc.tile_pool(name="psS", bufs=2, space="PSUM") as psS,
            tc.tile_pool(name="psC", bufs=2, space="PSUM") as psC,
            tc.tile_pool(name="psM", bufs=2, space="PSUM") as psM,
        ):
            # ---- constants -------------------------------------------------
            wq_sb = constp.tile([128, KC, 128], F32R, tag="wq")
            wk_sb = constp.tile([128, KC, 128], F32R, tag="wk")
            wv_sb = constp.tile([128, KC, 128], F32R, tag="wv")
            wo_sb = constp.tile([128, KC, 128], F32R, tag="wo")
            for kc in range(KC):
                nc.sync.dma_start(wq_sb[:, kc, :], wq_d[kc])
                nc.sync.dma_start(wk_sb[:, kc, :], wk_d[kc])
                nc.sync.dma_start(wv_sb[:, kc, :], wv_d[kc])
            nc.sync.dma_start(wo_sb[:], wo_d[:])
            swp_sb = constp.tile([128, 128], F32R, tag="swp")
            nc.sync.dma_start(swp_sb[:], swp_d[:])
            cos_sb = constp.tile([128, L], F32, tag="cos")
            sin_sb = constp.tile([128, L], F32, tag="sin")
            nc.sync.dma_start(cos_sb[:], cos_d[:])
            nc.sync.dma_start(sin_sb[:], sin_d[:])
            ident_sb = constp.tile([128, 128], F32, tag="ident")
            nc.sync.dma_start(ident_sb[:], ident_d[:])
            bqkv_sb = constp.tile([128, 3], F32, tag="bqkv")
            nc.sync.dma_start(bqkv_sb[:], bqkv_d[:])
            bo_sb = constp.tile([128, KC], F32, tag="bo")
            nc.sync.dma_start(bo_sb[:], bo_d[:])
            ones_sb = constp.tile([128, NKT], F32, tag="ones")
            nc.gpsimd.memset(ones_sb[:], 1.0)

            # o_proj task queue: emitted interleaved with later phases so the
            # PE always has independent work during attention chain stalls
            pending_o = []
            eplg_flip = [0]

            def _emit_oproj(task):
                bb, qb, mt, ctx2 = task
                ps = psM.tile([128, 512], F32, tag="m")
                nc.tensor.matmul(ps[:], wo_sb[:, mt, :], ctx2[:],
                                 start=True, stop=True)
                osb = outp.tile([128, 512], F32, tag="osb")
                if eplg_flip[0] % 2 == 0:
                    nc.vector.tensor_scalar_add(osb[:], ps[:],
                                                bo_sb[:, mt:mt + 1])
                else:
                    nc.scalar.activation(osb[:], ps[:],
                                         mybir.ActivationFunctionType.Identity,
                                         bias=bo_sb[:, mt:mt + 1])
                eplg_flip[0] += 1
                nc.sync.dma_start(out_d[bb, mt, :, qb * 512:(qb + 1) * 512],
                                  osb[:])

            def _drain_oproj(n):
                for _ in range(min(n, len(pending_o))):
                    _emit_oproj(pending_o.pop(0))

            for b in range(B):
                # ---- load x^T chunks --------------------------------------
                x_sb = []
                for kc in range(KC):
                    xt_t = xp.tile([128, L], F32R, tag="x")
                    nc.sync.dma_start(xt_t[:], xt_d[b, kc])
                    x_sb.append(xt_t)

                # ---- projections ------------------------------------------
                # lt in pairs so each weight chunk feeds 2 back-to-back
                # matmuls (amortizes the fp32 LDWEIGHTS).
                q_raw = qkvp.tile([128, L], F32R, tag="qraw")
                k_raw = qkvp.tile([128, L], F32R, tag="kraw")
                vT_raw = qkvp.tile([128, L], F32, tag="vraw")
                with nc.named_scope(f"proj_b{b}"):
                    for m, (w_sb, raw, bcol) in enumerate(
                        [(wq_sb, q_raw, 0), (wk_sb, k_raw, 1), (wv_sb, vT_raw, 2)]
                    ):
                        for lt in range(LT):
                            ps = psM.tile([128, 512], F32, tag="m")
                            for kc in range(KC):
                                nc.tensor.matmul(
                                    ps[:], w_sb[:, kc, :],
                                    x_sb[kc][:, lt * 512:(lt + 1) * 512],
                                    start=(kc == 0), stop=(kc == KC - 1),
                                )
                            nc.scalar.activation(
                                raw[:, lt * 512:(lt + 1) * 512], ps[:],
                                mybir.ActivationFunctionType.Identity,
                                bias=bqkv_sb[:, bcol:bcol + 1],
                            )
                            if b == 1:
                                _drain_oproj(1)

                # ---- RoPE on q, k -----------------------------------------
                q_rope = ropep.tile([128, L], F32R, tag="qrope")
                k_rope = ropep.tile([128, L], F32R, tag="krope")
                for raw, rope in [(q_raw, q_rope), (k_raw, k_rope)]:
                    for lt in range(LT):
                        sl = slice(lt * 512, (lt + 1) * 512)
                        ps = psM.tile([128, 512], F32, tag="m")
                        nc.tensor.matmul(ps[:], swp_sb[:], raw[:, sl],
                                         start=True, stop=True)
                        t1 = workp.tile([128, 512], F32, tag="t1")
                        nc.vector.tensor_mul(t1[:], raw[:, sl], cos_sb[:, sl])
                        t2 = workp.tile([128, 512], F32, tag="t2")
                        nc.vector.tensor_mul(t2[:], ps[:], sin_sb[:, sl])
                        nc.vector.tensor_add(rope[:, sl], t1[:], t2[:])

                # ---- V transpose to (l, dh) + ones column -----------------
                v_sb = vsbp.tile([128, NKT, 130], F32R, tag="v")
                nc.vector.tensor_copy(v_sb[:, :, 64], ones_sb[:])
                nc.vector.tensor_copy(v_sb[:, :, 129], ones_sb[:])
                for kt in range(NKT):
                    ps = psM.tile([128, 128], F32, tag="m")
                    nc.tensor.transpose(
                        ps[:], vT_raw[:, kt * 128:(kt + 1) * 128], ident_sb[:])
                    dst = v_sb[:, kt, :].rearrange("p (a c) -> p a c", a=2)
                    nc.vector.tensor_copy(
                        dst[:, :, 0:64],
                        ps[:].rearrange("p (a c) -> p a c", a=2))

                # ---- attention per q-block --------------------------------
                for qb in range(NQB):
                    qsl = slice(qb * 512, (qb + 1) * 512)
                    nkt = 4 * qb + 4
                    ctxA = psC.tile([65, 512], F32, tag="ctx")
                    ctxB = psC.tile([65, 512], F32, tag="ctx")
                    with nc.named_scope(f"attn_b{b}q{qb}"):
                        # software pipeline: scores(kt)+exp(kt) issue ahead of
                        # PV(kt-1) so the PE streams scores while ACT runs exp
                        p2s = {}
                        for kt in range(nkt + 1):
                            if kt < nkt:
                                ksl = slice(kt * 128, (kt + 1) * 128)
                                psAB = psS.tile([128, 1024], F32, tag="s")
                                nc.tensor.matmul(psAB[:, 0:512],
                                                 k_rope[0:64, ksl],
                                                 q_rope[0:64, qsl],
                                                 start=True, stop=True)
                                nc.tensor.matmul(psAB[:, 512:1024],
                                                 k_rope[64:128, ksl],
                                                 q_rope[64:128, qsl],
                                                 start=True, stop=True)
                                p2 = pp.tile([128, 2, 512], F32R, tag="p")
                                nc.scalar.activation(
                                    p2[:].rearrange("p a c -> p (a c)"), psAB[:],
                                    mybir.ActivationFunctionType.Exp,
                                    scale=0.125)
                                if kt >= 4 * qb:  # diag: keep jq >= jk + 128*v
                                    v = kt - 4 * qb
                                    nc.gpsimd.affine_select(
                                        out=p2[:], in_=p2[:],
                                        compare_op=mybir.AluOpType.is_ge,
                                        fill=0.0, base=-(128 * v),
                                        pattern=[[0, 2], [1, 512]],
                                        channel_multiplier=-1)
                                p2s[kt] = p2
                            if kt >= 1:
                                pv = kt - 1
                                p2v = p2s.pop(pv)
                                nc.tensor.matmul(ctxA[:], v_sb[:, pv, 0:65],
                                                 p2v[:, 0, :],
                                                 start=(pv == 0),
                                                 stop=(pv == nkt - 1))
                                nc.tensor.matmul(ctxB[:], v_sb[:, pv, 65:130],
                                                 p2v[:, 1, :],
                                                 start=(pv == 0),
                                                 stop=(pv == nkt - 1))
                            _drain_oproj(1)
                    # normalize: ctx2[h] = ctx[h][0:64] * (1/ctx[h][64])
                    ctx2 = c2p.tile([128, 512], F32R, tag="c2")
                    with nc.named_scope(f"norm_b{b}q{qb}"):
                        for h, ctx in enumerate((ctxA, ctxB)):
                            ssum = workp.tile([1, 512], F32, tag="ssum")
                            nc.scalar.copy(ssum[:], ctx[64:65, :])
                            rcp = workp.tile([1, 512], F32, tag="rcp")
                            nc.vector.reciprocal_approx_fast(rcp[:], ssum[:])
                            rb = workp.tile([64, 512], F32, tag="rb")
                            nc.gpsimd.partition_broadcast(rb[:], rcp[:])
                            nc.vector.tensor_mul(
                                ctx2[h * 64:(h + 1) * 64, :], ctx[0:64, :], rb[:])
                    pending_o.extend((b, qb, mt, ctx2) for mt in range(KC))
            # tail: drain remaining o_proj tasks (last q-block of batch 1)
            with nc.named_scope("oproj_tail"):
                _drain_oproj(len(pending_o))
    nc.compile()
    return nc


def _host_prep(x, qkv_w, qkv_b, o_w, o_b):
    """Build per-core input maps (all host-side reshapes/transposes)."""
    xt = np.ascontiguousarray(x.transpose(0, 2, 1)).reshape(B, KC, 128, L)

    half = DH // 2
    freq = 1.0 / (10000.0 ** (2.0 * np.arange(half, dtype=np.float64) / DH))
    t = np.arange(L, dtype=np.float64)
    freqs = t[:, None] * freq[None, :]                  # (L, 32)
    sinT = np.sin(freqs).T.astype(np.float32)           # (32, L)
    cosT = np.cos(freqs).T.astype(np.float32)
    cos128 = np.tile(cosT, (4, 1))                      # same for all 4 groups
    # signed sin: rows 0-31 (q1' = q1*cos - q2*sin) get -sin; 32-63 get +sin
    sin128 = np.concatenate([-sinT, sinT, -sinT, sinT], axis=0)

    # swap permutation: out partition p reads in partition p+32 (p%64<32) else p-32
    swp = np.zeros((128, 128), dtype=np.float32)
    for p in range(128):
        src = p + 32 if (p % 64) < 32 else p - 32
        swp[src, p] = 1.0

    ident = np.eye(128, dtype=np.float32)

    in_maps = []
    for c in range(NCORES):
        r = slice(128 * c, 128 * (c + 1))
        wq = np.ascontiguousarray(qkv_w[r].T).reshape(KC, 128, 128)
        wk = np.ascontiguousarray(qkv_w[D:][r].T).reshape(KC, 128, 128)
        wv = np.ascontiguousarray(qkv_w[2 * D:][r].T).reshape(KC, 128, 128)
        wo = np.ascontiguousarray(o_w[:, r].T).reshape(128, KC, 128)
        bqkv = np.stack([qkv_b[r], qkv_b[D:][r], qkv_b[2 * D:][r]],
                        axis=1).astype(np.float32)      # (128, 3)
        # o_b applied by core 0 only (host sums the row-parallel partials)
        if c == 0:
            bo = np.ascontiguousarray(o_b.reshape(KC, 128).T)  # (128, KC)
        else:
            bo = np.zeros((128, KC), dtype=np.float32)
        in_maps.append({
            "xt": xt, "wq": wq, "wk": wk, "wv": wv, "wo": wo,
            "swp": swp, "cosT": cos128, "sinT": sin128, "ident": ident,
            "bqkv": bqkv, "bo": bo,
        })
    return in_maps


def kernel(x, qkv_w, qkv_b, o_w, o_b, attn_mask, _trace=False):
    global _BUILT
    x = np.asarray(x, dtype=np.float32)
    qkv_w = np.asarray(qkv_w, dtype=np.float32)
    qkv_b = np.asarray(qkv_b, dtype=np.float32)
    o_w = np.asarray(o_w, dtype=np.float32)
    o_b = np.asarray(o_b, dtype=np.float32)
    # attn_mask is all-ones for this problem (spec fill=ones); causal handled
    # on device.

    if _BUILT is None:
        _BUILT = build()
    nc = _BUILT
    in_maps = _host_prep(x, qkv_w, qkv_b, o_w, o_b)
    res = run_bass_kernel_spmd(nc, in_maps, core_ids=list(range(NCORES)),
                               trace=_trace)
    # gather: sum row-parallel partials, then (B, KC, 128, L) -> (B, L, D)
    acc = np.zeros((B, KC, 128, L), dtype=np.float64)
    for r in res.results:
        acc += r["out"].astype(np.float64)
    out = acc.reshape(B, D, L).transpose(0, 2, 1).astype(np.float32)
    if _trace:
        return out, res
    return out

